# revision 94
# baseline (speedup 1.0000x reference)
"""Adaptive token pruner (entropy-gated cascaded db4 DWT) on 8 TRN2 NeuronCores.

Strategy (pure data parallel, 32 samples/core):
  - Each core receives its 32-sample shard of x as a flat row tensor and
    the FULL cls_attention_map (rotated so its own 32 samples are rows
    0..31); it computes all 256 entropies locally, so the batch-wide
    quantile needs no collective. Quantile thresholds reduce to rank
    comparisons (q25/q50 interpolation lies strictly between order stats
    63/64 and 127/128): level masks are m1 = rank>=128, m12 = rank>=64
    with rank[b] = #{j : s[j] > s[b]}, s[b] = sum_n a*ln(a+1e-9).
  - The 1/2/3-level lowpass DWT cascade along the 196 patch tokens is a
    banded matmul with seq as the contraction dim: y_sel = M_b^T @ patch,
    where M_b = D0 + m12*D1 + m1*D2 blends precomputed composite filter
    matrices (D0=C3p, D1=C2p-C3p, D2=W1-C2p); zero padding of shorter
    levels falls out exactly (blended columns are exact zeros).
  - x rows are row-PAIR packed (partition p holds rows 2p/2p+1, p<98:
    6KB DMA descriptors, K=98 contraction split by row parity); fp32r
    matmuls (full PE rate, ~FP22 precision => rel err ~1e-4); per sample
    4 matmuls + one PSUM->SBUF copy (DVE/ACT alternating).
  - Output stores are padded to EXACTLY 128 partitions per sample
    (contiguous 384KB): sub-128-partition SBUF->DRAM DMAs make the SWDGE
    broadcast one descriptor list to all 16 engines with per-entry skip
    walks (~15ns x descs x 16 engines of pure overhead) and serialize on
    an unbalanced engine subset; 128-partition stores descriptor-map
    cleanly like loads. Pad rows are ignored by the host gather; CLS
    tokens pass through from the input on the host.
"""

import numpy as np

import concourse.bass as bass
import concourse.mybir as mybir
import concourse.tile as tile
from concourse import bacc
from concourse.bass_utils import run_bass_kernel_spmd
from concourse.masks import make_identity

F32 = mybir.dt.float32
F32R = mybir.dt.float32r
BF16 = mybir.dt.bfloat16
FP16 = mybir.dt.float16
AF = mybir.ActivationFunctionType
ALU = mybir.AluOpType

B, N, D = 256, 197, 768
NCORES = 8
BPC = B // NCORES          # 32 samples per core
NPAT = N - 1               # 196 patch tokens
L1, L2, L3 = 101, 54, 30   # DWT output lengths
TOUT = 1 + L1              # 102 output rows per sample
XROWS = BPC * N            # flat x rows per core
KP = 98                    # partitions per sample load (2 rows each = 196)
G = 4                      # samples per DMA group

DB4_DEC_LO = np.array([
    -0.010597401784997278, 0.032883011666982945,
    0.030841381835986965, -0.18703481171888114,
    -0.02798376941698385, 0.6308807679295904,
    0.7148465705525415, 0.23037781330885523], dtype=np.float64)


def _build_w(n_in):
    """Banded matrix W (n_in, out_len) with y = W.T @ x equal to one level of
    zero-mode stride-2 db4 lowpass DWT (pytorch_wavelets conv semantics)."""
    h = DB4_DEC_LO[::-1]
    L = h.shape[0]
    out_len = (n_in + L - 1) // 2
    p = 2 * (out_len - 1) - n_in + L
    half = p // 2
    W = np.zeros((n_in, out_len), dtype=np.float64)
    for t in range(out_len):
        for l in range(L):
            n2 = 2 * t + l - half
            if 0 <= n2 < n_in:
                W[n2, t] += h[l]
    return W


def _build_dmat():
    """(98, 3, 2, 101) f32: D[p, m, q, t] = Dm[2*p + q, t] (row-PARITY split:
    SBUF partition p holds x rows 2p/2p+1, p<98 covers all 196 patch rows;
    the matmuls contract K=98)."""
    W1 = _build_w(NPAT)              # (196, 101)
    W2 = _build_w(L1)                # (101, 54)
    W3 = _build_w(L2)                # (54, 30)
    C2 = W1 @ W2
    C3 = C2 @ W3
    C2p = np.zeros((NPAT, L1)); C2p[:, :L2] = C2
    C3p = np.zeros((NPAT, L1)); C3p[:, :L3] = C3
    Ds = np.stack([C3p, C2p - C3p, W1 - C2p])      # (3, 196, 101)
    # parity split: [m, q, p, t] = Ds[m, 2p+q, t]
    Dq = Ds.reshape(3, KP, 2, L1).transpose(0, 2, 1, 3)    # (3, 2, 98, 101)
    return np.ascontiguousarray(Dq.transpose(2, 0, 1, 3)).astype(np.float32)


DMAT = _build_dmat()

_NC_CACHE = {}


def _strided(ap, offset, dims):
    c = ap.copy()
    c.ap = c.ap[:0] + [list(d) for d in dims]
    c.offset = offset
    return c


def _build_nc(init_pads=False):
    # init_pads writes the staging pad rows (101..127) so CoreSim's
    # uninitialized-read checker is satisfied; hardware doesn't need it
    # (the pad rows are ignored by the host gather).
    nc = bacc.Bacc("TRN2", target_bir_lowering=False, debug=False,
                   num_devices=NCORES)
    x = nc.dram_tensor("x", [XROWS, D], F32, kind="ExternalInput")
    attn = nc.dram_tensor("attn", [B, NPAT], F32, kind="ExternalInput")
    dmat = nc.dram_tensor("dmat", [KP, 3, 2, L1], F32, kind="ExternalInput")
    out = nc.dram_tensor("out", [BPC * 128, D], F32, kind="ExternalOutput")
    levels_out = nc.dram_tensor("levels", [1, BPC], F32, kind="ExternalOutput")

    # Constants initialized in a raw-bass preamble (barrier'd) so Tile ops
    # reading them carry no semaphore waits.
    ident_t = nc.alloc_sbuf_tensor("ident_c", [128, 128], F32)
    biast_t = nc.alloc_sbuf_tensor("biast_c", [128, 1], F32)
    ones_t = nc.alloc_sbuf_tensor("ones_c", [1, 128], F32)
    htop_t = nc.alloc_sbuf_tensor("htop_c", [1, 128], F32)
    hbot_t = nc.alloc_sbuf_tensor("hbot_c", [1, 128], F32)
    pre_sem = nc.alloc_semaphore("pre_sem")
    nc.gpsimd.memset(ident_t.ap(), 0.0).then_inc(pre_sem, 1)
    nc.gpsimd.wait_ge(pre_sem, 1)
    make_identity(nc, ident_t.ap(), nomemset=True)
    nc.gpsimd.memset(biast_t.ap(), 1e-9)
    nc.gpsimd.memset(ones_t.ap(), 1.0)
    nc.gpsimd.memset(htop_t.ap(), 0.0).then_inc(pre_sem, 1)
    nc.gpsimd.memset(hbot_t.ap(), 0.0).then_inc(pre_sem, 1)
    nc.gpsimd.wait_ge(pre_sem, 3)
    nc.gpsimd.memset(htop_t.ap()[0:1, 0:64], 1.0)
    nc.gpsimd.memset(hbot_t.ap()[0:1, 64:128], 1.0)
    nc.all_engine_barrier()
    ident = ident_t.ap()
    biast = biast_t.ap()
    ones_row = ones_t.ap()
    htop = htop_t.ap()
    hbot = hbot_t.ap()

    from contextlib import ExitStack
    with tile.TileContext(nc) as tc, ExitStack() as ctx:
        const = ctx.enter_context(tc.tile_pool(name="const", bufs=1))
        ent = ctx.enter_context(tc.tile_pool(name="ent", bufs=1))
        xpool = ctx.enter_context(tc.tile_pool(name="xin", bufs=5))
        mbpool = ctx.enter_context(tc.tile_pool(name="mb", bufs=16))
        tmpool = ctx.enter_context(tc.tile_pool(name="mbtmp", bufs=2))
        stg = ctx.enter_context(tc.tile_pool(name="stg", bufs=3))
        ppool = ctx.enter_context(tc.tile_pool(name="psum", bufs=4, space="PSUM"))
        pps = ppool  # prologue psum tiles share the main pool slots

        # ---- constants ----
        dm = const.tile([KP, 3, 2, L1], F32)
        nc.sync.dma_start(dm[:], dmat.ap())

        # ---- entropy -> rank -> level masks prologue ----
        at = ent.tile([128, 2, NPAT], F32)
        nc.sync.dma_start(at[:], attn.ap().rearrange("(j p) d -> p j d", p=128))
        lg = ent.tile([128, 2, NPAT], F32)
        junk0 = ent.tile([128, NPAT], F32)
        junk1 = ent.tile([128, NPAT], F32)
        s_col = ent.tile([128, 2], F32)
        # absorber: pull the attn-DMA wait onto the DVE engine clock so the
        # table-lowered STT ops below carry at most one embedded wait
        absorb = ent.tile([128, 1], F32)
        nc.vector.tensor_copy(absorb[:], at[:, 0, 0:1])
        for j, junk in ((0, junk0), (1, junk1)):
            nc.scalar.activation(lg[:, j, :], at[:, j, :], AF.Ln,
                                 bias=biast[:])
            nc.vector.scalar_tensor_tensor(
                out=junk[:], in0=lg[:, j, :], scalar=1.0, in1=at[:, j, :],
                op0=ALU.mult, op1=ALU.mult, accum_out=s_col[:, j:j + 1])
        # s values of all 256 samples as one broadcast row
        s_row = ent.tile([1, B], F32)
        for j in range(2):
            tp = pps.tile([1, 128], F32, tag="ps")
            nc.tensor.transpose(tp[:], s_col[:, j:j + 1], ident[:])
            nc.vector.tensor_copy(s_row[:, j * 128:(j + 1) * 128], tp[:])
        # broadcast s_row to all 128 partitions via ones (K=1) matmul
        s_bc = ent.tile([128, B], F32)
        bc_ps = pps.tile([128, B], F32, tag="ps")
        nc.tensor.matmul(bc_ps[:], ones_row[:], s_row[:], start=True, stop=True)
        nc.vector.tensor_copy(s_bc[:], bc_ps[:])
        # rank for tile-0 samples (rows 0..31 are this core's own samples)
        cmp = ent.tile([128, B], F32)
        rank = ent.tile([128, 1], F32)
        nc.vector.tensor_scalar(
            out=cmp[:], in0=s_bc[:], scalar1=s_col[:, 0:1], scalar2=None,
            op0=ALU.is_gt, op1=ALU.add, accum_out=rank[:])
        mtile = ent.tile([128, 3], F32)   # cols: m1, m12, level
        nc.vector.tensor_scalar(
            out=mtile[:, 0:1], in0=rank[:], scalar1=128.0, scalar2=None,
            op0=ALU.is_ge)
        nc.vector.tensor_scalar(
            out=mtile[:, 1:2], in0=rank[:], scalar1=64.0, scalar2=None,
            op0=ALU.is_ge)
        # level = 3 - m1 - m12
        nc.vector.scalar_tensor_tensor(
            out=mtile[:, 2:3], in0=mtile[:, 0:1], scalar=-1.0,
            in1=mtile[:, 1:2], op0=ALU.mult, op1=ALU.subtract)
        nc.vector.tensor_scalar(
            out=mtile[:, 2:3], in0=mtile[:, 2:3], scalar1=3.0, scalar2=None,
            op0=ALU.add)
        mrow = ent.tile([1, 2 * BPC], F32)
        lev_row = ent.tile([1, BPC], F32)
        for col, dst in ((0, mrow[:, 0:BPC]), (1, mrow[:, BPC:2 * BPC]),
                         (2, lev_row[:])):
            tpm = pps.tile([1, 128], F32, tag="ps")
            nc.tensor.transpose(tpm[:], mtile[:, col:col + 1], ident[:])
            nc.vector.tensor_copy(dst, tpm[0:1, 0:BPC])
        nc.sync.dma_start(levels_out.ap(), lev_row[:])
        m_bc = ent.tile([128, 2 * BPC], F32)   # [:, i]=m1_i, [:, 32+i]=m12_i
        mb_ps = pps.tile([128, 2 * BPC], F32, tag="ps")
        nc.tensor.matmul(mb_ps[:], ones_row[:], mrow[:], start=True, stop=True)
        nc.vector.tensor_copy(m_bc[:], mb_ps[:])

        # ---- main loop: blended banded matmul per sample ----
        NG = BPC // G
        LOOKAHEAD = 4

        def load_group(g):
            # partition p holds x rows (2p, 2p+1) per sample: one DMA per
            # group, 6KB contiguous DRAM runs per (partition, sample)
            b0 = g * G
            xt = xpool.tile([KP, G, 2, D], F32R, tag="xt")
            nc.gpsimd.dma_start(
                xt[:],
                _strided(x.ap(), (N * b0 + 1) * D,
                         [[2 * D, KP], [N * D, G], [1, 2 * D]]))
            return xt

        xts = {g: load_group(g) for g in range(LOOKAHEAD)}
        for g in range(NG):
            b0 = g * G
            xt = xts.pop(g)
            st = stg.tile([128, G, D], F32)
            if init_pads:
                nc.gpsimd.memset(st[96:128, :, :], 0.0)
            for s in range(G):
                i = b0 + s
                mb = mbpool.tile([KP, 2, L1], F32R)
                tmpb = tmpool.tile([KP, 2, L1], F32, tag="tmpb")
                nc.vector.scalar_tensor_tensor(
                    out=tmpb[:], in0=dm[:, 1, :, :],
                    scalar=m_bc[0:KP, BPC + i:BPC + i + 1],
                    in1=dm[:, 0, :, :], op0=ALU.mult, op1=ALU.add)
                nc.vector.scalar_tensor_tensor(
                    out=mb[:], in0=dm[:, 2, :, :],
                    scalar=m_bc[0:KP, i:i + 1],
                    in1=tmpb[:], op0=ALU.mult, op1=ALU.add)
                ps = ppool.tile([L1, D], F32, tag="ps")
                for j in range(2):
                    for n0, n1 in ((0, 512), (512, D)):
                        nc.tensor.matmul(
                            ps[:, n0:n1],
                            mb[:, j, :],
                            xt[:, s, j, n0:n1],
                            start=(j == 0), stop=(j == 1))
                if s % 2 == 0:
                    nc.vector.tensor_copy(st[0:L1, s, :], ps[:])
                else:
                    nc.scalar.copy(st[0:L1, s, :], ps[:])
                    # full-128-partition contiguous stores, 2 samples per
                    # DMA (rows L1..127 are padding the host ignores) —
                    # the exact-128-partition shape avoids the SWDGE
                    # broadcast+skip walk that sub-128 stores incur
                    nc.gpsimd.dma_start(
                        _strided(out.ap(), 128 * (b0 + s - 1) * D,
                                 [[D, 128], [128 * D, 2], [1, D]]),
                        st[:, s - 1:s + 1, :])
            if g + LOOKAHEAD < NG:
                xts[g + LOOKAHEAD] = load_group(g + LOOKAHEAD)
    nc.compile()
    return nc


def _get_nc():
    if "nc" not in _NC_CACHE:
        _NC_CACHE["nc"] = _build_nc()
    return _NC_CACHE["nc"]


def kernel(x, cls_attention_map, _bench_out=None):
    x = np.ascontiguousarray(np.asarray(x, dtype=np.float32))
    a = np.ascontiguousarray(np.asarray(cls_attention_map, dtype=np.float32))
    assert x.shape == (B, N, D) and a.shape == (B, NPAT)

    nc = _get_nc()
    in_maps = []
    for c in range(NCORES):
        xs = x[c * BPC:(c + 1) * BPC].reshape(BPC * N, D)
        at = np.concatenate([a[c * BPC:], a[:c * BPC]], axis=0)
        in_maps.append({"x": xs, "attn": at, "dmat": DMAT})

    trace = _bench_out is not None
    res = run_bass_kernel_spmd(nc, in_maps, core_ids=list(range(NCORES)),
                               trace=trace)
    if _bench_out is not None:
        _bench_out["results"] = res

    final = np.empty((B, TOUT, D), dtype=np.float32)
    mask = np.empty((B, TOUT), dtype=bool)
    final[:, 0, :] = x[:, 0, :]          # CLS tokens pass through unchanged
    for c in range(NCORES):
        yc = np.asarray(res.results[c]["out"]).reshape(BPC, 128, D)
        final[c * BPC:(c + 1) * BPC, 1:, :] = yc[:, :L1, :]
        lev = np.rint(np.asarray(res.results[c]["levels"])).astype(np.int64)
        lev = lev.reshape(BPC)
        out_len = np.where(lev == 1, L1, np.where(lev == 2, L2, L3))
        mask[c * BPC:(c + 1) * BPC, 0] = True
        mask[c * BPC:(c + 1) * BPC, 1:] = \
            np.arange(L1)[None, :] < out_len[:, None]
    return final, mask


# revision 99
# speedup vs baseline: 1.5514x; 1.5514x over previous
"""Adaptive token pruner (entropy-gated cascaded db4 DWT) on 8 TRN2 NeuronCores.

Strategy (pure data parallel, 32 samples/core):
  - Each core receives its 32-sample shard of x as a flat row tensor and
    the FULL cls_attention_map (rotated so its own 32 samples are rows
    0..31); it computes all 256 entropies locally, so the batch-wide
    quantile needs no collective. Quantile thresholds reduce to rank
    comparisons (q25/q50 interpolation lies strictly between order stats
    63/64 and 127/128): level masks are m1 = rank>=128, m12 = rank>=64
    with rank[b] = #{j : s[j] > s[b]}, s[b] = sum_n a*ln(a+1e-9).
  - The 1/2/3-level lowpass DWT cascade along the 196 patch tokens is a
    banded matmul with seq as the contraction dim: y_sel = M_b^T @ patch,
    where M_b = D0 + m12*D1 + m1*D2 blends precomputed composite filter
    matrices (D0=C3p, D1=C2p-C3p, D2=W1-C2p); zero padding of shorter
    levels falls out exactly (blended columns are exact zeros).
  - x rows are row-PAIR packed (partition p holds rows 2p/2p+1, p<98:
    6KB DMA descriptors, K=98 contraction split by row parity); fp32r
    matmuls (full PE rate, ~FP22 precision => rel err ~1e-4); per sample
    4 matmuls + one PSUM->SBUF copy (DVE/ACT alternating).
  - Output stores are padded to EXACTLY 128 partitions per sample
    (contiguous 384KB): sub-128-partition SBUF->DRAM DMAs make the SWDGE
    broadcast one descriptor list to all 16 engines with per-entry skip
    walks (~15ns x descs x 16 engines of pure overhead) and serialize on
    an unbalanced engine subset; 128-partition stores descriptor-map
    cleanly like loads. Pad rows are ignored by the host gather; CLS
    tokens pass through from the input on the host.
"""

import numpy as np

import concourse.bass as bass
import concourse.mybir as mybir
import concourse.tile as tile
from concourse import bacc
from concourse.bass_utils import run_bass_kernel_spmd
from concourse.masks import make_identity

F32 = mybir.dt.float32
F32R = mybir.dt.float32r
BF16 = mybir.dt.bfloat16
FP16 = mybir.dt.float16
AF = mybir.ActivationFunctionType
ALU = mybir.AluOpType

B, N, D = 256, 197, 768
NCORES = 8
BPC = B // NCORES          # 32 samples per core
NPAT = N - 1               # 196 patch tokens
L1, L2, L3 = 101, 54, 30   # DWT output lengths
TOUT = 1 + L1              # 102 output rows per sample
XROWS = BPC * N            # flat x rows per core
KP = 98                    # partitions per sample load (2 rows each = 196)
G = 4                      # samples per DMA group

DB4_DEC_LO = np.array([
    -0.010597401784997278, 0.032883011666982945,
    0.030841381835986965, -0.18703481171888114,
    -0.02798376941698385, 0.6308807679295904,
    0.7148465705525415, 0.23037781330885523], dtype=np.float64)


def _build_w(n_in):
    """Banded matrix W (n_in, out_len) with y = W.T @ x equal to one level of
    zero-mode stride-2 db4 lowpass DWT (pytorch_wavelets conv semantics)."""
    h = DB4_DEC_LO[::-1]
    L = h.shape[0]
    out_len = (n_in + L - 1) // 2
    p = 2 * (out_len - 1) - n_in + L
    half = p // 2
    W = np.zeros((n_in, out_len), dtype=np.float64)
    for t in range(out_len):
        for l in range(L):
            n2 = 2 * t + l - half
            if 0 <= n2 < n_in:
                W[n2, t] += h[l]
    return W


def _build_dmat():
    """(98, 3, 2, 101) f32: D[p, m, q, t] = Dm[2*p + q, t] (row-PARITY split:
    SBUF partition p holds x rows 2p/2p+1, p<98 covers all 196 patch rows;
    the matmuls contract K=98)."""
    W1 = _build_w(NPAT)              # (196, 101)
    W2 = _build_w(L1)                # (101, 54)
    W3 = _build_w(L2)                # (54, 30)
    C2 = W1 @ W2
    C3 = C2 @ W3
    C2p = np.zeros((NPAT, L1)); C2p[:, :L2] = C2
    C3p = np.zeros((NPAT, L1)); C3p[:, :L3] = C3
    Ds = np.stack([C3p, C2p - C3p, W1 - C2p])      # (3, 196, 101)
    # parity split: [m, q, p, t] = Ds[m, 2p+q, t]
    Dq = Ds.reshape(3, KP, 2, L1).transpose(0, 2, 1, 3)    # (3, 2, 98, 101)
    return np.ascontiguousarray(Dq.transpose(2, 0, 1, 3)).astype(np.float32)


DMAT = _build_dmat()

_NC_CACHE = {}


def _strided(ap, offset, dims):
    c = ap.copy()
    c.ap = c.ap[:0] + [list(d) for d in dims]
    c.offset = offset
    return c


def _build_nc(init_pads=False):
    # init_pads writes the staging pad rows (101..127) so CoreSim's
    # uninitialized-read checker is satisfied; hardware doesn't need it
    # (the pad rows are ignored by the host gather).
    nc = bacc.Bacc("TRN2", target_bir_lowering=False, debug=False,
                   num_devices=NCORES)
    x = nc.dram_tensor("x", [XROWS, D], F32, kind="ExternalInput")
    attn = nc.dram_tensor("attn", [B, NPAT], F32, kind="ExternalInput")
    dmat = nc.dram_tensor("dmat", [KP, 3, 2, L1], F32, kind="ExternalInput")
    out = nc.dram_tensor("out", [BPC * 128, D], FP16, kind="ExternalOutput")
    levels_out = nc.dram_tensor("levels", [1, BPC], F32, kind="ExternalOutput")

    # Constants initialized in a raw-bass preamble (barrier'd) so Tile ops
    # reading them carry no semaphore waits.
    ident_t = nc.alloc_sbuf_tensor("ident_c", [128, 128], F32)
    biast_t = nc.alloc_sbuf_tensor("biast_c", [128, 1], F32)
    ones_t = nc.alloc_sbuf_tensor("ones_c", [1, 128], F32)
    htop_t = nc.alloc_sbuf_tensor("htop_c", [1, 128], F32)
    hbot_t = nc.alloc_sbuf_tensor("hbot_c", [1, 128], F32)
    pre_sem = nc.alloc_semaphore("pre_sem")
    nc.gpsimd.memset(ident_t.ap(), 0.0).then_inc(pre_sem, 1)
    nc.gpsimd.wait_ge(pre_sem, 1)
    make_identity(nc, ident_t.ap(), nomemset=True)
    nc.gpsimd.memset(biast_t.ap(), 1e-9)
    nc.gpsimd.memset(ones_t.ap(), 1.0)
    nc.gpsimd.memset(htop_t.ap(), 0.0).then_inc(pre_sem, 1)
    nc.gpsimd.memset(hbot_t.ap(), 0.0).then_inc(pre_sem, 1)
    nc.gpsimd.wait_ge(pre_sem, 3)
    nc.gpsimd.memset(htop_t.ap()[0:1, 0:64], 1.0)
    nc.gpsimd.memset(hbot_t.ap()[0:1, 64:128], 1.0)
    nc.all_engine_barrier()
    ident = ident_t.ap()
    biast = biast_t.ap()
    ones_row = ones_t.ap()
    htop = htop_t.ap()
    hbot = hbot_t.ap()

    from contextlib import ExitStack
    with tile.TileContext(nc) as tc, ExitStack() as ctx:
        const = ctx.enter_context(tc.tile_pool(name="const", bufs=1))
        ent = ctx.enter_context(tc.tile_pool(name="ent", bufs=1))
        xpool = ctx.enter_context(tc.tile_pool(name="xin", bufs=5))
        mbpool = ctx.enter_context(tc.tile_pool(name="mb", bufs=16))
        tmpool = ctx.enter_context(tc.tile_pool(name="mbtmp", bufs=2))
        stg = ctx.enter_context(tc.tile_pool(name="stg", bufs=3))
        ppool = ctx.enter_context(tc.tile_pool(name="psum", bufs=4, space="PSUM"))
        pps = ppool  # prologue psum tiles share the main pool slots

        # ---- constants ----
        dm = const.tile([KP, 3, 2, L1], F32)
        nc.sync.dma_start(dm[:], dmat.ap())

        # ---- entropy -> rank -> level masks prologue ----
        at = ent.tile([128, 2, NPAT], F32)
        nc.sync.dma_start(at[:], attn.ap().rearrange("(j p) d -> p j d", p=128))
        lg = ent.tile([128, 2, NPAT], F32)
        junk0 = ent.tile([128, NPAT], F32)
        junk1 = ent.tile([128, NPAT], F32)
        s_col = ent.tile([128, 2], F32)
        # absorber: pull the attn-DMA wait onto the DVE engine clock so the
        # table-lowered STT ops below carry at most one embedded wait
        absorb = ent.tile([128, 1], F32)
        nc.vector.tensor_copy(absorb[:], at[:, 0, 0:1])
        for j, junk in ((0, junk0), (1, junk1)):
            nc.scalar.activation(lg[:, j, :], at[:, j, :], AF.Ln,
                                 bias=biast[:])
            nc.vector.scalar_tensor_tensor(
                out=junk[:], in0=lg[:, j, :], scalar=1.0, in1=at[:, j, :],
                op0=ALU.mult, op1=ALU.mult, accum_out=s_col[:, j:j + 1])
        # s values of all 256 samples as one broadcast row
        s_row = ent.tile([1, B], F32)
        for j in range(2):
            tp = pps.tile([1, 128], F32, tag="ps")
            nc.tensor.transpose(tp[:], s_col[:, j:j + 1], ident[:])
            nc.vector.tensor_copy(s_row[:, j * 128:(j + 1) * 128], tp[:])
        # broadcast s_row to all 128 partitions via ones (K=1) matmul
        s_bc = ent.tile([128, B], F32)
        bc_ps = pps.tile([128, B], F32, tag="ps")
        nc.tensor.matmul(bc_ps[:], ones_row[:], s_row[:], start=True, stop=True)
        nc.vector.tensor_copy(s_bc[:], bc_ps[:])
        # rank for tile-0 samples (rows 0..31 are this core's own samples)
        cmp = ent.tile([128, B], F32)
        rank = ent.tile([128, 1], F32)
        nc.vector.tensor_scalar(
            out=cmp[:], in0=s_bc[:], scalar1=s_col[:, 0:1], scalar2=None,
            op0=ALU.is_gt, op1=ALU.add, accum_out=rank[:])
        mtile = ent.tile([128, 3], F32)   # cols: m1, m12, level
        nc.vector.tensor_scalar(
            out=mtile[:, 0:1], in0=rank[:], scalar1=128.0, scalar2=None,
            op0=ALU.is_ge)
        nc.vector.tensor_scalar(
            out=mtile[:, 1:2], in0=rank[:], scalar1=64.0, scalar2=None,
            op0=ALU.is_ge)
        # level = 3 - m1 - m12
        nc.vector.scalar_tensor_tensor(
            out=mtile[:, 2:3], in0=mtile[:, 0:1], scalar=-1.0,
            in1=mtile[:, 1:2], op0=ALU.mult, op1=ALU.subtract)
        nc.vector.tensor_scalar(
            out=mtile[:, 2:3], in0=mtile[:, 2:3], scalar1=3.0, scalar2=None,
            op0=ALU.add)
        mrow = ent.tile([1, 2 * BPC], F32)
        lev_row = ent.tile([1, BPC], F32)
        for col, dst in ((0, mrow[:, 0:BPC]), (1, mrow[:, BPC:2 * BPC]),
                         (2, lev_row[:])):
            tpm = pps.tile([1, 128], F32, tag="ps")
            nc.tensor.transpose(tpm[:], mtile[:, col:col + 1], ident[:])
            nc.vector.tensor_copy(dst, tpm[0:1, 0:BPC])
        nc.sync.dma_start(levels_out.ap(), lev_row[:])
        m_bc = ent.tile([128, 2 * BPC], F32)   # [:, i]=m1_i, [:, 32+i]=m12_i
        mb_ps = pps.tile([128, 2 * BPC], F32, tag="ps")
        nc.tensor.matmul(mb_ps[:], ones_row[:], mrow[:], start=True, stop=True)
        nc.vector.tensor_copy(m_bc[:], mb_ps[:])

        # ---- main loop: blended banded matmul per sample ----
        NG = BPC // G
        LOOKAHEAD = 4

        def load_group(g):
            # partition p holds x rows (2p, 2p+1) per sample: one DMA per
            # group, 6KB contiguous DRAM runs per (partition, sample)
            b0 = g * G
            xt = xpool.tile([KP, G, 2, D], FP16, tag="xt")
            nc.gpsimd.dma_start(
                xt[:],
                _strided(x.ap(), (N * b0 + 1) * D,
                         [[2 * D, KP], [N * D, G], [1, 2 * D]]))
            return xt

        xts = {g: load_group(g) for g in range(LOOKAHEAD)}
        for g in range(NG):
            b0 = g * G
            xt = xts.pop(g)
            st = stg.tile([128, G, D], FP16)
            if init_pads:
                nc.gpsimd.memset(st[96:128, :, :], 0.0)
            for s in range(G):
                i = b0 + s
                mb = mbpool.tile([KP, 2, L1], FP16)
                tmpb = tmpool.tile([KP, 2, L1], F32, tag="tmpb")
                nc.vector.scalar_tensor_tensor(
                    out=tmpb[:], in0=dm[:, 1, :, :],
                    scalar=m_bc[0:KP, BPC + i:BPC + i + 1],
                    in1=dm[:, 0, :, :], op0=ALU.mult, op1=ALU.add)
                nc.vector.scalar_tensor_tensor(
                    out=mb[:], in0=dm[:, 2, :, :],
                    scalar=m_bc[0:KP, i:i + 1],
                    in1=tmpb[:], op0=ALU.mult, op1=ALU.add)
                ps = ppool.tile([L1, D], F32, tag="ps")
                for j in range(2):
                    for n0, n1 in ((0, 512), (512, D)):
                        nc.tensor.matmul(
                            ps[:, n0:n1],
                            mb[:, j, :],
                            xt[:, s, j, n0:n1],
                            start=(j == 0), stop=(j == 1))
                if s % 2 == 0:
                    nc.vector.tensor_copy(st[0:L1, s, :], ps[:])
                else:
                    nc.scalar.copy(st[0:L1, s, :], ps[:])
                # full-128-partition contiguous store (rows L1..127 are
                # padding the host ignores) — the exact-128-partition
                # shape avoids the SWDGE broadcast+skip walk that
                # sub-128-partition SBUF->DRAM stores incur
                nc.gpsimd.dma_start(
                    _strided(out.ap(), 128 * (b0 + s) * D,
                             [[D, 128], [1, D]]),
                    st[:, s, :])
            if g + LOOKAHEAD < NG:
                xts[g + LOOKAHEAD] = load_group(g + LOOKAHEAD)
    nc.compile()
    return nc


def _get_nc():
    if "nc" not in _NC_CACHE:
        _NC_CACHE["nc"] = _build_nc()
    return _NC_CACHE["nc"]


def kernel(x, cls_attention_map, _bench_out=None):
    x = np.ascontiguousarray(np.asarray(x, dtype=np.float32))
    a = np.ascontiguousarray(np.asarray(cls_attention_map, dtype=np.float32))
    assert x.shape == (B, N, D) and a.shape == (B, NPAT)

    nc = _get_nc()
    in_maps = []
    for c in range(NCORES):
        xs = x[c * BPC:(c + 1) * BPC].reshape(BPC * N, D)
        at = np.concatenate([a[c * BPC:], a[:c * BPC]], axis=0)
        in_maps.append({"x": xs, "attn": at, "dmat": DMAT})

    trace = _bench_out is not None
    res = run_bass_kernel_spmd(nc, in_maps, core_ids=list(range(NCORES)),
                               trace=trace)
    if _bench_out is not None:
        _bench_out["results"] = res

    final = np.empty((B, TOUT, D), dtype=np.float32)
    mask = np.empty((B, TOUT), dtype=bool)
    final[:, 0, :] = x[:, 0, :]          # CLS tokens pass through unchanged
    for c in range(NCORES):
        yc = np.asarray(res.results[c]["out"]).reshape(BPC, 128, D)
        final[c * BPC:(c + 1) * BPC, 1:, :] = yc[:, :L1, :]
        lev = np.rint(np.asarray(res.results[c]["levels"])).astype(np.int64)
        lev = lev.reshape(BPC)
        out_len = np.where(lev == 1, L1, np.where(lev == 2, L2, L3))
        mask[c * BPC:(c + 1) * BPC, 0] = True
        mask[c * BPC:(c + 1) * BPC, 1:] = \
            np.arange(L1)[None, :] < out_len[:, None]
    return final, mask
